# revision 1
# baseline (speedup 1.0000x reference)
"""Trainium2 Bass kernel for nn_CNN_Casual (LeNet-ish CNN, B=8192).

Pure data parallel over 8 NeuronCores: 1024 samples per core, parameters
replicated, one SPMD Bass program. Per core, samples are processed in
blocks of 128 (the TensorEngine stationary-operand width):

  conv1  : the host gathers x into overlapping windows (8 input rows x 16
           cols = K 128) and folds sigmoid(mask) into a per-window Toeplitz
           weight matrix [128, 480] (exact - the mask is elementwise on the
           input and conv is linear). Per (row-window, col-half): one fp16
           matmul, stationary = data [128, 128 samples], moving = weights
           [128, 480 = 4 output rows x 10 ch x 12 cols].
  pool1  : fused 2x2 max of the PSUM tile. Split between a DVE-direct
           6D-AP reduce_max (XY over the pair dims) and an ACT psum->fp16
           copy + two DVE fp16 tensor_max stages (2x_1P mode), chosen per
           tile to balance the two engines (GPSIMD cannot run TT/reduce
           through walrus, and cannot read PSUM).
  T1     : PE transposes (fp16, 1 cyc/row) into a shared [120, 512] PSUM
           tile; one relu(x + b1) eviction per 4 transposes (ScalarE
           activation or DVE scalar_tensor_tensor, alternating). The
           per-channel conv bias commutes with max-pool so it is applied
           here, where it is per-PARTITION (free on the eviction op).
  conv2  : Toeplitz master [120, 7*160] = [Z,W4,W3,W2,W1,W0,Z] in fp16;
           output-row-pair group g accumulates 6 uniform-width (N=320)
           matmuls in PSUM; zero blocks keep every matmul wide enough to
           hide the weight load and make has_written semantics uniform.
  pool2/T2: same pattern -> f_all [80, 1024] (fp16) per 256-sample pair.
  fc1    : weights stationary [80, 50] x 4 groups, moving = f slices
           [80, 2x128]; relu+bias -> fc1o [50, 256] fp16.
  fc2    : data stationary [50, 128], moving weights [50, 10].
  softmax: per block, DVE computes t1 = (logits - rowmax) + fc2_b (any
           per-sample shift is exact for log_softmax); a half-core batched
           epilogue does one Exp, one windowed reduce_sum, one Ln and the
           final subtracts, so the ScalarE activation table loads once.

dtypes: conv inputs/weights and pooled activations are fp16 (|x| <= ~30,
11-bit mantissa keeps the end-to-end max relative error ~4e-4 vs the fp32
reference); PSUM accumulation is always fp32; pooling/softmax arithmetic
is fp32 except where noted. DMA: one input DMA per 256 samples (512B
runs), weights ~1.9MB once, one output DMA per 512 samples.
"""

from contextlib import ExitStack

import numpy as np

import concourse.mybir as mybir
import concourse.tile as tile
from concourse import bacc
from concourse.bass_utils import run_bass_kernel_spmd

F32 = mybir.dt.float32
FP16 = mybir.dt.float16
AF = mybir.ActivationFunctionType
AX = mybir.AxisListType

N_CORES = 8
B_TOTAL = 8192
B_CORE = B_TOTAL // N_CORES  # 1024


# --------------------------------------------------------------------------
# Host-side weight preparation (tiny tensors; exact rearrangement only)
# --------------------------------------------------------------------------
def _prep_weights(mask_w, conv1_w, conv1_b, conv2_w, conv2_b, fc1_w, fc1_b,
                  fc2_w, fc2_b):
    f32 = np.float32
    sig = (1.0 / (1.0 + np.exp(-mask_w.astype(f32)))).astype(f32)  # [28,28]

    # conv1 Toeplitz windows with mask folded in.
    # window (w,h): input rows 4w..4w+7, cols 12h..12h+15 (K = 8*16 = 128)
    # col index of the moving matrix: dp*120 + o*12 + ql
    #   (output row p = 4w+dp, output col q = 12h+ql)
    w1b = np.zeros((128, 480), f32)
    oo = np.arange(10)
    for dp in range(4):
        for ki in range(5):
            i = dp + ki
            for kj in range(5):
                for ql in range(12):
                    j = ql + kj
                    w1b[i * 16 + j, dp * 120 + oo * 12 + ql] = \
                        conv1_w[:, 0, ki, kj]
    w1m = np.empty((12, 128, 480), np.float16)
    for w in range(6):
        for h in range(2):
            win = sig[4 * w:4 * w + 8, 12 * h:12 * h + 16].reshape(128, 1)
            w1m[w * 2 + h] = (w1b * win).astype(np.float16)
    w1m = np.ascontiguousarray(w1m.transpose(1, 0, 2).reshape(128, 5760))

    # conv2 master Toeplitz: blocks [Z, W4, W3, W2, W1, W0, Z], each [120,160]
    # row index (c, j) = c*12 + j; col index (o2, q2) = o2*8 + q2
    w2m = np.zeros((120, 7, 160), np.float16)
    o2 = np.arange(20)
    for k in range(5):
        blk = 5 - k
        for c in range(10):
            for kj in range(5):
                for q2 in range(8):
                    j = q2 + kj
                    w2m[c * 12 + j, blk, o2 * 8 + q2] = conv2_w[:, c, k, kj]
    w2m_flat = np.ascontiguousarray(w2m.reshape(120, 7 * 160))

    # fc1 weights per pooled-row group p': rows (o2, s2), torch flatten order
    # of the conv2 activations is (o2, p', s2).
    fc1w4 = fc1_w.reshape(50, 20, 4, 4)  # [m, o2, p', s2]
    wfc1 = np.concatenate(
        [np.ascontiguousarray(fc1w4[:, :, p, :].reshape(50, 80).T)
         for p in range(4)],
        axis=1,
    )  # [80, 200]

    # const blob 1 (fp32): ident | bc2 | b1 | b2 | bf1  -> [128, 141]
    cst = np.zeros((128, 141), f32)
    cst[:, 0:128] = np.eye(128, dtype=f32)
    # constant stabilizing shift for log_softmax (exact: any per-sample
    # constant cancels); logits stay well inside fp32 exp range
    cst[:, 128:138] = np.tile(fc2_b.astype(f32).reshape(1, 10) - 10.0,
                              (128, 1))
    cst[0:120, 138] = np.repeat(conv1_b.astype(f32), 12)
    cst[0:80, 139] = np.repeat(conv2_b.astype(f32), 4)
    cst[0:50, 140] = fc1_b.astype(f32)

    # const blob 2 (fp16): fc2_w.T | wfc1 -> [80, 210]
    wfcb = np.zeros((80, 210), np.float16)
    wfcb[0:50, 0:10] = fc2_w.T.astype(np.float16)
    wfcb[:, 10:210] = wfc1.astype(np.float16)

    idb = np.eye(128).astype(np.float16)
    return dict(w1m=w1m, w2m=w2m_flat, wfcb=wfcb, cst=cst, idb=idb)


# --------------------------------------------------------------------------
# Device program
# --------------------------------------------------------------------------
def _build(b_core):
    assert b_core % 256 == 0
    n_pair = b_core // 256

    nc = bacc.Bacc("TRN2", target_bir_lowering=False, debug=False,
                   num_devices=N_CORES)

    xw_d = nc.dram_tensor("xw", [12, 128, b_core], FP16,
                          kind="ExternalInput").ap()
    w1m_d = nc.dram_tensor("w1m", [128, 5760], FP16,
                           kind="ExternalInput").ap()
    w2m_d = nc.dram_tensor("w2m", [120, 1120], FP16, kind="ExternalInput").ap()
    wfcb_d = nc.dram_tensor("wfcb", [80, 210], FP16, kind="ExternalInput").ap()
    cst_d = nc.dram_tensor("cst", [128, 141], F32, kind="ExternalInput").ap()
    idb_d = nc.dram_tensor("idb", [128, 128], FP16, kind="ExternalInput").ap()
    y = nc.dram_tensor("y", [b_core, 10], F32, kind="ExternalOutput").ap()

    with tile.TileContext(nc) as tc, ExitStack() as ctx:
        consts = ctx.enter_context(tc.tile_pool(name="consts", bufs=1))
        identb = consts.tile([128, 128], FP16)
        nc.sync.dma_start(identb[:], idb_d)
        w1m_sb = consts.tile([128, 5760], FP16)
        w2m_sb = consts.tile([120, 1120], FP16)
        wfcb_sb = consts.tile([80, 210], FP16)
        cst_sb = consts.tile([128, 141], F32)

        ident = cst_sb[:, 0:128]
        bc2_sb = cst_sb[:, 128:138]
        b1_sb = cst_sb[0:120, 138:139]
        b2_sb = cst_sb[0:80, 139:140]
        bf1_sb = cst_sb[0:50, 140:141]
        wfc2_sb = wfcb_sb[0:50, 0:10]
        wfc1_sb = wfcb_sb[:, 10:210]

        zeros = consts.tile([120, 512], FP16)
        nc.vector.memset(zeros[:], 0.0)

        xw_pool = ctx.enter_context(tc.tile_pool(name="xw", bufs=3))
        ps1_pool = ctx.enter_context(tc.tile_pool(name="ps1", bufs=3,
                                                  space="PSUM"))
        tmp_pool = ctx.enter_context(tc.tile_pool(name="tmpb", bufs=6))
        prp_pool = ctx.enter_context(tc.tile_pool(name="prp", bufs=4))
        tpw_pool = ctx.enter_context(tc.tile_pool(name="tpw", bufs=2,
                                                  space="PSUM"))
        x2_pool = ctx.enter_context(tc.tile_pool(name="x2", bufs=2))
        ps2_pool = ctx.enter_context(tc.tile_pool(name="ps2", bufs=2,
                                                  space="PSUM"))
        psf_pool = ctx.enter_context(tc.tile_pool(name="psf", bufs=1,
                                                  space="PSUM"))
        f_pool = ctx.enter_context(tc.tile_pool(name="fp", bufs=2))
        fc1o_pool = ctx.enter_context(tc.tile_pool(name="fc1o", bufs=2))
        sm_pool = ctx.enter_context(tc.tile_pool(name="sm", bufs=3))
        t1_all = consts.tile([128, 10 * 2 * n_pair], F32)

        SUB, ADD, MAX = (mybir.AluOpType.subtract, mybir.AluOpType.add,
                         mybir.AluOpType.max)

        def relu_bias_evict(idx, dst, src_ps, bias, width):
            """dst = relu(src_ps + bias) rounded to f32r; alternate engines."""
            if idx % 2 == 0:
                nc.scalar.activation(dst, src_ps, AF.Relu, bias=bias)
            else:
                nc.vector.scalar_tensor_tensor(
                    dst, src_ps, bias, zeros[:dst.shape[0], :width],
                    op0=ADD, op1=MAX)

        for pair in range(n_pair):
            f_all = f_pool.tile([80, 1024], FP16, name="f_all", tag="f_all")
            fview = f_all.rearrange("p (h g n) -> p g h n", h=2, g=4, n=128)
            xwcat = xw_pool.tile([128, 3072], FP16, name="xwcat", tag="xw")
            deng = nc.sync if pair % 2 == 0 else nc.scalar
            deng.dma_start(
                xwcat.rearrange("p (t n) -> p t n", t=12),
                xw_d[:, :, pair * 256:pair * 256 + 256]
                .rearrange("t p n -> p t n"))
            for half in range(2):
                blk = pair * 2 + half
                b0 = blk * 128
                prp_t = []
                for w in range(6):
                    prp = prp_pool.tile([128, 240], FP16, name="prp_t",
                                        tag="prp")
                    prp_t.append(prp)
                    for h in range(2):
                        t = w * 2 + h
                        if pair == 0 and half == 0:
                            eng = nc.sync if t % 2 == 0 else nc.scalar
                            eng.dma_start(
                                w1m_sb[:, t * 480:(t + 1) * 480],
                                w1m_d[:, t * 480:(t + 1) * 480])
                        ps1 = ps1_pool.tile([128, 480], F32, name="ps1_t",
                                            tag="ps1")
                        nc.tensor.matmul(ps1[:],
                                         xwcat[:, t * 256 + half * 128:
                                               t * 256 + half * 128 + 128],
                                         w1m_sb[:, t * 480:(t + 1) * 480],
                                         start=True, stop=True)
                        # pool 2x2: reduce over (tr, tc) of
                        # [p, u, o, m, tr, tc]; dst strided into prp
                        dst = prp.rearrange("p (u o q) -> p u o q",
                                            u=2, o=10)[:, :, :, 6 * h:6 * h + 6]
                        if t in (0, 3, 6, 9):
                            src = ps1.rearrange(
                                "p (u tr o m tc) -> p u o m tr tc",
                                u=2, tr=2, o=10, m=6)
                            nc.vector.reduce_max(dst, src, axis=AX.XY)
                        else:
                            tmp = tmp_pool.tile([128, 480], FP16,
                                                name="tmpb_t", tag="tmpb")
                            nc.scalar.copy(tmp[:], ps1[:])
                            tv = tmp.rearrange("p (u tr c) -> p u tr c",
                                               u=2, tr=2)
                            rm = tmp_pool.tile([128, 240], FP16,
                                               name="rm_t", tag="rm")
                            rmv = rm.rearrange("p (u c) -> p u c", u=2)
                            nc.vector.tensor_max(rmv, tv[:, :, 0],
                                                 tv[:, :, 1])
                            rv = rm.rearrange("p (u o m tc) -> p u o m tc",
                                              u=2, o=10, m=6)
                            nc.vector.tensor_max(dst, rv[:, :, :, :, 0],
                                                 rv[:, :, :, :, 1])
                if pair == 0 and half == 0:
                    nc.scalar.dma_start(cst_sb[:], cst_d)
                    nc.sync.dma_start(w2m_sb[:], w2m_d)
                    nc.scalar.dma_start(wfcb_sb[:], wfcb_d)
                # ---- T1 transposes into wide psum + relu/bias evict ----
                x2cat = []
                for ww in range(3):
                    tpw = tpw_pool.tile([120, 512], FP16, name="tpw_t",
                                        tag="tpw")
                    for q in range(2):
                        prp = prp_t[ww * 2 + q]
                        for u in range(2):
                            nc.tensor.transpose(
                                tpw[:, (q * 2 + u) * 128:
                                    (q * 2 + u + 1) * 128],
                                prp[:, u * 120:u * 120 + 120], identb[:])
                    x2c = x2_pool.tile([120, 512], FP16, name="x2c_t",
                                       tag=f"x2c{ww}")
                    relu_bias_evict(ww + blk, x2c[:], tpw[:],
                                    b1_sb[:, 0:1], 512)
                    x2cat.append(x2c)
                # ---- conv2 + pool2 + T2 + evict ----
                tp2w = tpw_pool.tile([80, 512], FP16, name="tp2w_t", tag="tpw")
                for g in range(4):
                    ps2g = ps2_pool.tile([128, 320], F32,
                                         name=f"ps2_{g}", tag="ps2")
                    for d in range(6):
                        r = 2 * g + d
                        lhsT = x2cat[r // 4][:, (r % 4) * 128:
                                             (r % 4 + 1) * 128]
                        nc.tensor.matmul(ps2g[:], lhsT,
                                         w2m_sb[:, (5 - d) * 160:
                                                (7 - d) * 160],
                                         start=(d == 0), stop=(d == 5))
                        if d == 5:
                            p2 = prp_pool.tile([128, 80], FP16, name="p2_t",
                                               tag="p2")
                            p2v = p2.rearrange("p (o s) -> p o s", o=20)
                            if g % 2 == 0:
                                src = ps2g.rearrange(
                                    "p (pl o s tc) -> p o s pl tc",
                                    pl=2, o=20, s=4)
                                nc.vector.reduce_max(p2v, src, axis=AX.XY)
                            else:
                                tmp2 = tmp_pool.tile([128, 320], FP16,
                                                     name="tmp2_t", tag="tmp2")
                                nc.scalar.copy(tmp2[:], ps2g[:])
                                t2v = tmp2.rearrange("p (pl c) -> p pl c",
                                                     pl=2)
                                rm2 = tmp_pool.tile([128, 160], FP16,
                                                    name="rm2_t", tag="rm2")
                                nc.vector.tensor_max(rm2[:], t2v[:, 0],
                                                     t2v[:, 1])
                                r2v = rm2.rearrange(
                                    "p (o s tc) -> p o s tc", o=20, s=4)
                                nc.vector.tensor_max(p2v, r2v[:, :, :, 0],
                                                     r2v[:, :, :, 1])
                            nc.tensor.transpose(
                                tp2w[:, g * 128:(g + 1) * 128], p2[:],
                                identb[:])
                relu_bias_evict(blk, f_all[:, half * 512:half * 512 + 512],
                                tp2w[:], b2_sb[:, 0:1], 512)
            # ---- fc1 over the 256-sample pair ----
            psf1 = psf_pool.tile([50, 256], F32, name="psf1", tag="psf")
            for g in range(4):
                nc.tensor.matmul(psf1[:], wfc1_sb[:, g * 50:(g + 1) * 50],
                                 fview[:, g], start=(g == 0), stop=(g == 3))
            fc1o = fc1o_pool.tile([50, 256], FP16, name="fc1o", tag="fc1o")
            nc.scalar.activation(fc1o[:], psf1[:], AF.Relu,
                                 bias=bf1_sb[:, 0:1])
            # ---- fc2 + stabilized shift (log_softmax epilogue is batched) --
            for half in range(2):
                blk = pair * 2 + half
                psf2 = psf_pool.tile([128, 10], F32, name="psf2", tag="psf")
                nc.tensor.matmul(psf2[:],
                                 fc1o[:, half * 128:half * 128 + 128],
                                 wfc2_sb[:], start=True, stop=True)
                # t1 = psf2 + (fc2_b - 10): a constant shift is exact for
                # log_softmax and keeps exp() comfortably in fp32 range
                nc.vector.tensor_add(t1_all[:, blk * 10:blk * 10 + 10],
                                     psf2[:], bc2_sb[:])
            # ---- batched log_softmax epilogue, one half-core at a time ----
            if pair % (max(n_pair // 2, 1)) == max(n_pair // 2, 1) - 1:
                hb = 2 * (pair + 1 - max(n_pair // 2, 1))  # first blk of half
                nb = 2 * max(n_pair // 2, 1)
                c0 = hb * 10
                tslice = t1_all[:, c0:c0 + 10 * nb]
                e_all = sm_pool.tile([128, 10 * nb], F32, name="e_all",
                                     tag="e_all")
                nc.scalar.activation(e_all[:], tslice, AF.Exp)
                se = sm_pool.tile([128, nb], F32, name="se", tag="se")
                nc.vector.reduce_sum(
                    se[:], e_all.rearrange("p (b t) -> p b t", t=10),
                    axis=AX.X)
                ls = sm_pool.tile([128, nb], F32, name="ls", tag="ls")
                nc.scalar.activation(ls[:], se[:], AF.Ln)
                yo = sm_pool.tile([128, 10 * nb], F32, name="yo", tag="yo")
                for b in range(nb):
                    nc.vector.tensor_scalar_sub(
                        yo[:, b * 10:b * 10 + 10],
                        t1_all[:, (hb + b) * 10:(hb + b) * 10 + 10],
                        ls[:, b:b + 1])
                nc.scalar.dma_start(
                    y[hb * 128:(hb + nb) * 128]
                    .rearrange("(blk p) c -> p blk c", p=128),
                    yo.rearrange("p (blk c) -> p blk c", c=10))

    nc.compile()
    return nc


_PROGRAM_CACHE = {}


def _get_program(b_core):
    if b_core not in _PROGRAM_CACHE:
        _PROGRAM_CACHE[b_core] = _build(b_core)
    return _PROGRAM_CACHE[b_core]


def make_in_maps(x, weights, b_core=B_CORE, n_cores=N_CORES):
    """Shard x over cores; replicate the (rearranged) parameters."""
    f32 = np.float32
    xr = np.asarray(x, dtype=f32).reshape(-1, 28, 28)
    in_maps = []
    for c in range(n_cores):
        xc = xr[c * b_core:(c + 1) * b_core]  # [b_core, 28, 28]
        xwin = np.empty((12, 128, b_core), np.float16)
        for w in range(6):
            for h in range(2):
                win = xc[:, 4 * w:4 * w + 8, 12 * h:12 * h + 16]
                xwin[w * 2 + h] = win.reshape(b_core, 128).T
        m = {"xw": np.ascontiguousarray(xwin)}
        m.update(weights)
        in_maps.append(m)
    return in_maps


def kernel(**inputs):
    x = np.asarray(inputs["x"], dtype=np.float32)
    weights = _prep_weights(
        np.asarray(inputs["mask_w"], np.float32),
        np.asarray(inputs["conv1_w"], np.float32),
        np.asarray(inputs["conv1_b"], np.float32),
        np.asarray(inputs["conv2_w"], np.float32),
        np.asarray(inputs["conv2_b"], np.float32),
        np.asarray(inputs["fc1_w"], np.float32),
        np.asarray(inputs["fc1_b"], np.float32),
        np.asarray(inputs["fc2_w"], np.float32),
        np.asarray(inputs["fc2_b"], np.float32),
    )
    nc = _get_program(B_CORE)
    in_maps = make_in_maps(x, weights)
    res = run_bass_kernel_spmd(nc, in_maps, list(range(N_CORES)))
    out = np.concatenate([res.results[c]["y"] for c in range(N_CORES)], axis=0)
    return np.ascontiguousarray(out.astype(np.float32))


if __name__ == "__main__":
    rng = np.random.default_rng(0)
    ins = {
        "x": rng.standard_normal((B_TOTAL, 1, 28, 28), dtype=np.float32),
        "mask_w": rng.standard_normal((28, 28), dtype=np.float32) * 0.1,
        "conv1_w": rng.standard_normal((10, 1, 5, 5), dtype=np.float32) * 0.2,
        "conv1_b": rng.standard_normal((10,), dtype=np.float32) * 0.1,
        "conv2_w": rng.standard_normal((20, 10, 5, 5), dtype=np.float32) * 0.06,
        "conv2_b": rng.standard_normal((20,), dtype=np.float32) * 0.1,
        "fc1_w": rng.standard_normal((50, 320), dtype=np.float32) * 0.05,
        "fc1_b": rng.standard_normal((50,), dtype=np.float32) * 0.1,
        "fc2_w": rng.standard_normal((10, 50), dtype=np.float32) * 0.14,
        "fc2_b": rng.standard_normal((10,), dtype=np.float32) * 0.1,
    }
    out = kernel(**ins)
    print(out.shape, out.dtype, out[:2])



# revision 5
# speedup vs baseline: 1.0827x; 1.0827x over previous
"""Trainium2 Bass kernel for nn_CNN_Casual (LeNet-ish CNN, B=8192). v2.

Pure data parallel over 8 NeuronCores: 1024 samples/core, 8 blocks of 128.

Key structure (vs baseline):
  conv1  : fp8-e4m3 DoubleRow matmuls, data-corrected: stationary holds
           (x_hi, x_lo) planes (x = x_hi + x_lo, both e4m3; exact to ~2^-8),
           moving holds the masked Toeplitz weights (e4m3, scale S1)
           duplicated into both pair slots.  2 windows share one 2-bank
           psum tile; 2 chunk matmuls per window at 0.5 cycles/column.
           conv1 bias rides the sacrificial lo-slot row 127 (corner pixel
           lo-bits dropped; ~0.3% of one tap).
  pool1  : per window-pair, either
             M_D: one fused 6D reduce_max (4:1) from psum (DVE only), or
             M_A: ACT relu-copy psum->fp16 + two packed-fp16 2x tensor_max
           chosen per-wpair to balance DVE vs ACT.
  T1     : PE transposes into [120,512] psum; ACT Relu evict -> x2cat fp16
           [121, 512] (row 120 = ones for the conv2 bias row).
  conv2  : fp16, single-output-row accumulation: 8 rows x 5 matmuls
           [121 x 160], bias via the ones row (b2 in W row 120 of ki=0).
  pool2  : per row-pair: fused 4D reduce_max (M_D) or ACT copy + TT (M_A).
  T2/fc1 : transpose -> f_all [81, 1024] (ones row 80 -> fc1 bias);
           fc1o = ACT Relu(psf1), fp16.
  fc2    : psf2 [128, 10]; DVE add of f32 (fc2_b - 10) -> t1_all.
  softmax: one batched epilogue per core: Exp(80) + 6D reduce_sum + Ln +
           8 scalar subs; single output DMA.  Act tables load twice total.

dtypes: fp8 only where corrected (conv1 data path); fp16 elsewhere on PE;
fp32 PSUM + f32 fc2 bias add keep log_softmax exact to ~1.6e-2 max rel.
"""

from contextlib import ExitStack

import numpy as np
import ml_dtypes

import concourse.mybir as mybir
import concourse.tile as tile
from concourse import bacc
from concourse.bass_utils import run_bass_kernel_spmd

F32 = mybir.dt.float32
FP16 = mybir.dt.float16
FP8 = mybir.dt.float8e4
AF = mybir.ActivationFunctionType
AX = mybir.AxisListType
DR = mybir.MatmulPerfMode.DoubleRow
E4 = ml_dtypes.float8_e4m3

N_CORES = 8
B_TOTAL = 8192
B_CORE = B_TOTAL // N_CORES  # 1024
S1 = 32.0

# pool mode tables, tuned against TimelineSim: 'A' = ACT-copy path,
# 'D' = DVE direct reduce.  MODE1: 6 window-pairs/block; MODE2: 4 row-pairs.
MODE1 = ["ADADAD", "ADAADA"]  # alternating per block parity
MODE2 = ["DDDD", "DDDD"]


# --------------------------------------------------------------------------
# Host-side weight preparation
# --------------------------------------------------------------------------
def _q8(a):
    return a.astype(E4)


def _prep_weights(mask_w, conv1_w, conv1_b, conv2_w, conv2_b, fc1_w, fc1_b,
                  fc2_w, fc2_b):
    f32 = np.float32
    sig = (1.0 / (1.0 + np.exp(-mask_w.astype(f32)))).astype(f32)  # [28,28]

    # conv1 Toeplitz per window t=(w,h): [128, 480], n = (tr,tc,u,o,m)
    # out row p = 4w + 2u + tr, col q = 12h + 2m + tc; k = r*16 + c
    # value = conv1_w[o,0,ki,kj] * sig[p+ki, q+kj], ki=r-dp, kj=c-ql
    w1m = np.zeros((128, 12 * 960), E4)
    oo = np.arange(10)
    for w in range(6):
        for h in range(2):
            t = w * 2 + h
            wt = np.zeros((128, 480), f32)
            for u in range(2):
                for tr in range(2):
                    dp = 2 * u + tr
                    for m in range(6):
                        for tc in range(2):
                            ql = 2 * m + tc
                            for ki in range(5):
                                r = dp + ki
                                for kj in range(5):
                                    c = ql + kj
                                    n = tr * 240 + tc * 120 + u * 60 + oo * 6 + m
                                    wt[r * 16 + c, n] = (
                                        conv1_w[:, 0, ki, kj]
                                        * sig[4 * w + r, 12 * h + c])
            wq = _q8(wt * S1)
            slot1 = wq.copy()
            # bias value depends only on o: col n -> o = (n % 60) // 6
            nidx = np.arange(480)
            bias_n = conv1_b.astype(f32)[(nidx % 60) // 6] * S1
            slot1[127, :] = _q8(bias_n)
            for c2 in range(2):
                base = t * 960 + c2 * 480
                w1m[:, base:base + 240] = wq[:, c2 * 240:(c2 + 1) * 240]
                w1m[:, base + 240:base + 480] = slot1[:, c2 * 240:
                                                      (c2 + 1) * 240]

    # conv2 single-row Toeplitz: per ki [121, 160], n2 = tc*80 + o*4 + s
    # (q2 = 2s + tc); row (c*12 + j) = conv2_w[o,c,ki,j-q2]/S1; row 120 =
    # b2[o] for ki==0.
    w2m = np.zeros((121, 800), np.float16)
    for ki in range(5):
        blk = np.zeros((121, 160), f32)
        for c in range(10):
            for j in range(12):
                for o in range(20):
                    for s in range(4):
                        for tc in range(2):
                            q2 = 2 * s + tc
                            kj = j - q2
                            if 0 <= kj < 5:
                                blk[c * 12 + j, tc * 80 + o * 4 + s] = \
                                    conv2_w[o, c, ki, kj] / S1
        if ki == 0:
            o_of = (np.arange(160) % 80) // 4
            blk[120, :] = conv2_b.astype(f32)[o_of]
        w2m[:, ki * 160:(ki + 1) * 160] = blk.astype(np.float16)

    # fc1 weights per pooled-row group p': [81, 200]; row 80 = fc1_b (g0)
    fc1w4 = fc1_w.reshape(50, 20, 4, 4)  # [m, o2, p', s2]
    wfc1 = np.zeros((81, 200), np.float16)
    for p in range(4):
        wfc1[0:80, p * 50:(p + 1) * 50] = \
            fc1w4[:, :, p, :].reshape(50, 80).T.astype(np.float16)
    wfc1[80, 0:50] = fc1_b.astype(np.float16)

    # wfcb fp16 [81, 210]: fc2_w.T | wfc1
    wfcb = np.zeros((81, 210), np.float16)
    wfcb[0:50, 0:10] = fc2_w.T.astype(np.float16)
    wfcb[:, 10:210] = wfc1

    # cst f32 [128, 20]: doubled (fc2_b - 10)
    cst = np.tile(np.concatenate([fc2_b.astype(f32) - 10.0] * 2)
                  .reshape(1, 20), (128, 1)).astype(f32)

    idb = np.eye(128).astype(np.float16)
    return dict(w1m=w1m, w2m=w2m, wfcb=wfcb, cst=cst, idb=idb)


# --------------------------------------------------------------------------
# Device program
# --------------------------------------------------------------------------
def _build(b_core):
    assert b_core % 256 == 0
    n_pair = b_core // 256

    nc = bacc.Bacc("TRN2", target_bir_lowering=False, debug=False,
                   num_devices=N_CORES)

    xw_d = nc.dram_tensor("xw", [12, 128, 2 * b_core], FP8,
                          kind="ExternalInput").ap()
    w1m_d = nc.dram_tensor("w1m", [128, 11520], FP8,
                           kind="ExternalInput").ap()
    w2m_d = nc.dram_tensor("w2m", [121, 800], FP16, kind="ExternalInput").ap()
    wfcb_d = nc.dram_tensor("wfcb", [81, 210], FP16,
                            kind="ExternalInput").ap()
    cst_d = nc.dram_tensor("cst", [128, 20], F32, kind="ExternalInput").ap()
    idb_d = nc.dram_tensor("idb", [128, 128], FP16, kind="ExternalInput").ap()
    y = nc.dram_tensor("y", [b_core, 10], F32, kind="ExternalOutput").ap()

    MAX, ADD = mybir.AluOpType.max, mybir.AluOpType.add

    with tile.TileContext(nc) as tc, ExitStack() as ctx:
        consts = ctx.enter_context(tc.tile_pool(name="consts", bufs=1))
        identb = consts.tile([128, 128], FP16)
        nc.sync.dma_start(identb[:], idb_d)
        w1m_sb = consts.tile([128, 11520], FP8)
        w2m_sb = consts.tile([121, 800], FP16)
        wfcb_sb = consts.tile([81, 210], FP16)
        cst_sb = consts.tile([128, 20], F32)

        wfc2_sb = wfcb_sb[0:50, 0:10]
        wfc1_sb = wfcb_sb[:, 10:210]
        t1_all = consts.tile([128, 10 * 8], F32)

        xw_pool = ctx.enter_context(tc.tile_pool(name="xw", bufs=3))
        ps1_pool = ctx.enter_context(tc.tile_pool(name="ps1", bufs=2,
                                                  space="PSUM"))
        tmp_pool = ctx.enter_context(tc.tile_pool(name="tmpb", bufs=4))
        prp_pool = ctx.enter_context(tc.tile_pool(name="prp", bufs=8))
        tpw_pool = ctx.enter_context(tc.tile_pool(name="tpw", bufs=2,
                                                  space="PSUM"))
        x2_pool = ctx.enter_context(tc.tile_pool(name="x2", bufs=6))
        ps2_pool = ctx.enter_context(tc.tile_pool(name="ps2", bufs=2,
                                                  space="PSUM"))
        p2_pool = ctx.enter_context(tc.tile_pool(name="p2", bufs=5))
        f_pool = ctx.enter_context(tc.tile_pool(name="fp", bufs=2))
        fc1o_pool = ctx.enter_context(tc.tile_pool(name="fc1o", bufs=2))
        sm_pool = ctx.enter_context(tc.tile_pool(name="sm", bufs=1))

        for pair in range(n_pair):
            xwcat = xw_pool.tile([128, 12 * 512], FP8, name="xwcat", tag="xw")
            deng = nc.sync if pair % 2 == 0 else nc.scalar
            deng.dma_start(
                xwcat.rearrange("p (t c) -> p t c", t=12),
                xw_d[:, :, pair * 512:(pair + 1) * 512]
                .rearrange("t p c -> p t c"))

            f_all = f_pool.tile([81, 1024], FP16, name="f_all", tag="f_all")
            nc.gpsimd.memset(f_all[80:81, :], 1.0)

            for half in range(2):
                blk = pair * 2 + half
                mode1 = MODE1[blk % 2]
                mode2 = MODE2[blk % 2]

                # ---- conv1 (fp8 DR) + pool1, per window pair ----
                prp_t = []
                for wp in range(6):
                    pst = ps1_pool.tile([128, 1024], F32, name="pst",
                                        tag="ps1")
                    for wi in range(2):
                        t = wp * 2 + wi
                        if pair == 0 and half == 0:
                            eng = nc.sync if t % 2 == 0 else nc.scalar
                            eng.dma_start(
                                w1m_sb[:, t * 960:(t + 1) * 960],
                                w1m_d[:, t * 960:(t + 1) * 960])
                        lhsT = xwcat[:, t * 512 + half * 256:
                                     t * 512 + half * 256 + 256] \
                            .rearrange("p (two m) -> p two m", two=2)
                        for c2 in range(2):
                            nc.tensor.matmul(
                                pst[:, wi * 512 + c2 * 240:
                                    wi * 512 + c2 * 240 + 240],
                                lhsT,
                                w1m_sb[:, t * 960 + c2 * 480:
                                       t * 960 + (c2 + 1) * 480]
                                .rearrange("p (two f) -> p two f", two=2),
                                start=True, stop=True, perf_mode=DR)
                    # pooling: psum n-order per window = (tr, tc, u, o, m),
                    # windows at 512-elem offsets (480 used, 32 pad)
                    prp = prp_pool.tile([128, 240], FP16, name="prp",
                                        tag="prp")
                    prp_t.append(prp)
                    # prp col order (u, o, h, m): u:120, o:12, h:6, m:1
                    prp_v = prp.rearrange("p (u o h m) -> p h (u o) m",
                                          u=2, o=10, h=2)
                    pwin = pst.rearrange("p (h z) -> p h z", h=2)[:, :, 0:480]
                    if mode1[wp] == "D":
                        src6 = pwin.rearrange(
                            "p h (tr tc uo m) -> p h uo m tr tc",
                            tr=2, tc=2, uo=20)
                        nc.vector.reduce_max(prp_v, src6, axis=AX.XY)
                    else:
                        tmp = tmp_pool.tile([128, 960], FP16, name="tmp",
                                            tag="tmp")
                        nc.scalar.activation(
                            tmp.rearrange("p (h f) -> p h f", h=2), pwin,
                            AF.Relu)
                        tv = tmp.rearrange("p (h tr f) -> p h tr f",
                                           h=2, tr=2)
                        rm = tmp_pool.tile([128, 480], FP16, name="rm",
                                           tag="rm")
                        rmv = rm.rearrange("p (h f) -> p h f", h=2)
                        nc.vector.tensor_max(rmv, tv[:, :, 0], tv[:, :, 1])
                        rv5 = rm.rearrange("p (h tc uo m) -> p h tc uo m",
                                           h=2, tc=2, uo=20)
                        nc.vector.tensor_max(prp_v, rv5[:, :, 0],
                                             rv5[:, :, 1])

                # ---- T1 transposes + relu evict -> x2cat ----
                x2cat = []
                for ww in range(3):
                    tpw = tpw_pool.tile([120, 512], FP16, name="tpw",
                                        tag="tpw")
                    for q in range(4):
                        r = ww * 4 + q          # pooled row 0..11
                        prp = prp_t[r // 2]
                        u = r % 2
                        nc.tensor.transpose(
                            tpw[:, q * 128:(q + 1) * 128],
                            prp[:, u * 120:u * 120 + 120], identb[:])
                    x2c = x2_pool.tile([121, 512], FP16, name="x2c",
                                       tag="x2c")
                    nc.gpsimd.memset(x2c[120:121, :], 1.0)
                    nc.scalar.activation(x2c[0:120, :], tpw[:], AF.Relu)
                    x2cat.append(x2c)

                if pair == 0 and half == 0:
                    nc.sync.dma_start(w2m_sb[:], w2m_d)
                    nc.scalar.dma_start(wfcb_sb[:], wfcb_d)
                    nc.scalar.dma_start(cst_sb[:], cst_d)

                # ---- conv2 (fp16 single-row) + pool2 + T2 ----
                tp2w = tpw_pool.tile([80, 512], FP16, name="tp2w", tag="tpw")
                for g in range(4):
                    ps2 = ps2_pool.tile([128, 320], F32, name="ps2",
                                        tag="ps2")
                    for sub in range(2):
                        p2r = g * 2 + sub
                        for ki in range(5):
                            i = p2r + ki
                            nc.tensor.matmul(
                                ps2[:, sub * 160:sub * 160 + 160],
                                x2cat[i // 4][0:121,
                                              (i % 4) * 128:(i % 4 + 1) * 128],
                                w2m_sb[:, ki * 160:(ki + 1) * 160],
                                start=(ki == 0), stop=(ki == 4))
                    # pool2: region layout n2 = (tc, o, s); rows = pl
                    p2t = p2_pool.tile([128, 80], FP16, name="p2t", tag="p2t")
                    if mode2[g] == "D":
                        src = ps2.rearrange("p (pl tc os) -> p os pl tc",
                                            pl=2, tc=2)
                        nc.vector.reduce_max(p2t[:], src, axis=AX.XY)
                    else:
                        tmp2 = tmp_pool.tile([128, 320], FP16, name="tmp2",
                                             tag="tmp2")
                        nc.scalar.activation(tmp2[:], ps2[:], AF.Relu)
                        t2v = tmp2.rearrange("p (pl f) -> p pl f", pl=2)
                        rm2 = tmp_pool.tile([128, 160], FP16, name="rm2",
                                            tag="rm2")
                        nc.vector.tensor_max(rm2[:], t2v[:, 0], t2v[:, 1])
                        r2v = rm2.rearrange("p (tc f) -> p tc f", tc=2)
                        nc.vector.tensor_max(p2t[:], r2v[:, 0], r2v[:, 1])
                    nc.tensor.transpose(tp2w[:, g * 128:(g + 1) * 128],
                                        p2t[:], identb[:])
                nc.scalar.activation(f_all[0:80, half * 512:half * 512 + 512],
                                     tp2w[:], AF.Relu)

            # ---- fc1 over the 256-sample pair ----
            psf1 = ps2_pool.tile([50, 256], F32, name="psf1", tag="ps2")
            for g in range(4):
                rows = 81 if g == 0 else 80
                fvg = f_all[0:rows, :].rearrange("p (h g n) -> p g h n",
                                                 h=2, g=4)[:, g]
                nc.tensor.matmul(psf1[:], wfc1_sb[0:rows, g * 50:g * 50 + 50],
                                 fvg, start=(g == 0), stop=(g == 3))
            fc1o = fc1o_pool.tile([50, 256], FP16, name="fc1o", tag="fc1o")
            nc.scalar.activation(fc1o[:], psf1[:], AF.Relu)
            # ---- fc2 ----
            psf2 = ps2_pool.tile([128, 20], F32, name="psf2", tag="ps2")
            for half in range(2):
                nc.tensor.matmul(psf2[:, half * 10:half * 10 + 10],
                                 fc1o[:, half * 128:half * 128 + 128],
                                 wfc2_sb[:], start=True, stop=True)
            nc.vector.tensor_add(t1_all[:, pair * 20:pair * 20 + 20],
                                 psf2[:], cst_sb[:])

        # ---- batched log_softmax epilogue ----
        e_all = sm_pool.tile([128, 80], F32, name="e_all", tag="e_all")
        nc.scalar.activation(e_all[:], t1_all[:], AF.Exp)
        se = sm_pool.tile([128, 8], F32, name="se", tag="se")
        nc.vector.reduce_sum(se[:], e_all.rearrange("p (b t) -> p b t", t=10),
                             axis=AX.X)
        ls = sm_pool.tile([128, 8], F32, name="ls", tag="ls")
        nc.scalar.activation(ls[:], se[:], AF.Ln)
        yo = sm_pool.tile([128, 80], F32, name="yo", tag="yo")
        for b in range(8):
            nc.vector.tensor_scalar_sub(yo[:, b * 10:b * 10 + 10],
                                        t1_all[:, b * 10:b * 10 + 10],
                                        ls[:, b:b + 1])
        nc.scalar.dma_start(
            y.rearrange("(blk p) c -> p blk c", p=128),
            yo.rearrange("p (blk c) -> p blk c", c=10))

    nc.compile()
    return nc


_PROGRAM_CACHE = {}


def _get_program(b_core):
    if b_core not in _PROGRAM_CACHE:
        _PROGRAM_CACHE[b_core] = _build(b_core)
    return _PROGRAM_CACHE[b_core]


def make_in_maps(x, weights, b_core=B_CORE, n_cores=N_CORES):
    """Shard x over cores; replicate the (rearranged) parameters."""
    f32 = np.float32
    xr = np.asarray(x, dtype=f32).reshape(-1, 28, 28)
    in_maps = []
    for cidx in range(n_cores):
        xc = xr[cidx * b_core:(cidx + 1) * b_core]  # [b_core, 28, 28]
        xwin = np.zeros((12, 128, 2 * b_core), E4)
        for w in range(6):
            for h in range(2):
                t = w * 2 + h
                win = xc[:, 4 * w:4 * w + 8, 12 * h:12 * h + 16] \
                    .reshape(b_core, 128).astype(f32)
                hi = win.astype(E4)
                lo = (win - hi.astype(f32)).astype(E4)
                lo_f = lo.astype(f32)
                lo_f[:, 127] = 1.0
                lo = lo_f.astype(E4)
                hiT = hi.astype(f32).T.astype(E4)   # [128, b_core]
                loT = lo.astype(f32).T.astype(E4)
                for blk in range(b_core // 128):
                    p, hb = blk // 2, blk % 2
                    base = p * 512 + hb * 256
                    xwin[t, :, base:base + 128] = \
                        hiT[:, blk * 128:(blk + 1) * 128]
                    xwin[t, :, base + 128:base + 256] = \
                        loT[:, blk * 128:(blk + 1) * 128]
        m = {"xw": xwin}
        m.update(weights)
        in_maps.append(m)
    return in_maps


def kernel(**inputs):
    x = np.asarray(inputs["x"], dtype=np.float32)
    weights = _prep_weights(
        np.asarray(inputs["mask_w"], np.float32),
        np.asarray(inputs["conv1_w"], np.float32),
        np.asarray(inputs["conv1_b"], np.float32),
        np.asarray(inputs["conv2_w"], np.float32),
        np.asarray(inputs["conv2_b"], np.float32),
        np.asarray(inputs["fc1_w"], np.float32),
        np.asarray(inputs["fc1_b"], np.float32),
        np.asarray(inputs["fc2_w"], np.float32),
        np.asarray(inputs["fc2_b"], np.float32),
    )
    nc = _get_program(B_CORE)
    in_maps = make_in_maps(x, weights)
    res = run_bass_kernel_spmd(nc, in_maps, list(range(N_CORES)))
    out = np.concatenate([res.results[c]["y"] for c in range(N_CORES)],
                         axis=0)
    return np.ascontiguousarray(out.astype(np.float32))


if __name__ == "__main__":
    rng = np.random.default_rng(0)
    ins = {
        "x": rng.standard_normal((B_TOTAL, 1, 28, 28), dtype=np.float32),
        "mask_w": rng.standard_normal((28, 28), dtype=np.float32) * 0.1,
        "conv1_w": rng.standard_normal((10, 1, 5, 5), dtype=np.float32) * 0.2,
        "conv1_b": rng.standard_normal((10,), dtype=np.float32) * 0.1,
        "conv2_w": rng.standard_normal((20, 10, 5, 5),
                                       dtype=np.float32) * 0.06,
        "conv2_b": rng.standard_normal((20,), dtype=np.float32) * 0.1,
        "fc1_w": rng.standard_normal((50, 320), dtype=np.float32) * 0.05,
        "fc1_b": rng.standard_normal((50,), dtype=np.float32) * 0.1,
        "fc2_w": rng.standard_normal((10, 50), dtype=np.float32) * 0.14,
        "fc2_b": rng.standard_normal((10,), dtype=np.float32) * 0.1,
    }
    out = kernel(**ins)
    print(out.shape, out.dtype, out[:2])
